# revision 15
# baseline (speedup 1.0000x reference)
"""Trainium2 Bass kernel for nn_FCond (FiLM-conditioned MLP chain).

Reference computation (B=32, N=100000, D=3, CDIM=128):
    h = x
    for kblk in [0, 1, 2, 2, 2, 2]:
        h = tanh((h @ Wk.T + bk) * sigmoid(c @ Wsk.T + bsk) + (c @ Wbk.T + bbk))

Since the FiLM conditioning depends only on (c, weights), each (batch,
block) reduces to an affine map  h' = tanh(A_kb @ h + d_kb)  with
A_kb [3,3], d_kb [3] precomputed on the host in float64.

Device strategy (pure data parallel over 8 cores, 4 batches/core):
  - Layout: partition p = b*32 + comp*10 + g  (4 batch-bands of 32
    partitions; 3 comps x 10 point-groups per band; rows 30,31 of each
    band are zero padding). Free dim = 10240 points per (b,comp,g)
    stream (N padded 100000 -> 102400).
  - Each block is ONE block-diagonal [128x128] matmul on TensorE
    (40 real points per column), PSUM accumulated, then ScalarE does
    tanh(psum + d) with a per-partition bias AP, evacuating PSUM->SBUF.
  - 5 chunks of 2048 columns stream through DMA-in -> 6 blocks -> DMA-out
    with double buffering.

MM_DTYPE: float32r (TF32-like PE mode, 1 cyc/col) vs float32 (exact,
4 cyc/col). Selected by MM_EXACT below.
"""
import sys
import types

import numpy as np

B, N, D, CDIM = 32, 100000, 3, 128
NCORES = 8
BPC = B // NCORES          # batches per core
G = 10                     # point-groups per (batch, comp)
L = 10240                  # points per partition stream (N padded / G)
NPAD = G * L               # padded N = 102400
P = 128                    # partitions
CHUNK = 2048               # free-dim chunk (4 PSUM banks fp32)
MM_F = 512                 # matmul free chunk (1 PSUM bank)
NCHUNK = L // CHUNK

MM_EXACT = False           # True -> float32 matmuls (exact, ~2.9x slower PE)

PROFILE = False            # set by test harness; collects HW exec time
LAST_EXEC_NS = None

_CACHE = {}


def _install_profile_shim():
    """Register the NTFF profile hook (missing antenv.axon_hooks in this
    container) so run_bass_kernel_spmd(trace=True) can report exec time."""
    if "antenv.axon_hooks" in sys.modules:
        return
    mod = types.ModuleType("antenv.axon_hooks")
    _state = {"hook": None}
    mod.set_axon_ntff_profile_hook = lambda h: _state.__setitem__("hook", h)
    mod.get_axon_ntff_profile_hook = lambda: _state["hook"]
    sys.modules["antenv.axon_hooks"] = mod
    try:
        from trn_agent_boot.trn_boot import _ntff_profile_via_ctypes
        mod.set_axon_ntff_profile_hook(
            _ntff_profile_via_ctypes("/opt/axon/libaxon_pjrt.so"))
    except Exception:
        pass
    import concourse.bass_utils as bu
    bu.upload_artifacts = lambda tmpdir: f"local:{tmpdir}"


def _build_program():
    import concourse.bacc as bacc
    import concourse.tile as tile
    from concourse import mybir

    f32 = mybir.dt.float32
    mmdt = f32 if MM_EXACT else mybir.dt.float32r
    Tanh = mybir.ActivationFunctionType.Tanh
    Copy = mybir.ActivationFunctionType.Copy
    WSETS = (0, 1, 2, 2, 2, 2)

    nc = bacc.Bacc("TRN2", target_bir_lowering=False, debug=False)
    x_d = nc.declare_dram_parameter("x", [P, L], f32, isOutput=False)
    w_d = nc.declare_dram_parameter("w", [3, P, P], f32, isOutput=False)
    d_d = nc.declare_dram_parameter("d", [P, 3], f32, isOutput=False)
    y_d = nc.declare_dram_parameter("y", [P, L], f32, isOutput=True)

    with tile.TileContext(nc) as tc:
        with (
            tc.tile_pool(name="wpool", bufs=1) as wpool,
            tc.tile_pool(name="xinpool", bufs=5) as xinpool,
            tc.tile_pool(name="youtpool", bufs=1) as youtpool,
            tc.tile_pool(name="hpool", bufs=6) as hpool,
            tc.tile_pool(name="psum", bufs=2, space="PSUM") as psum,
        ):
            # --- first compute chunk's DMA goes out before anything else
            # so the PE/ACT chain can start ASAP. ---
            h0 = xinpool.tile([P, 512], mmdt, name="xin0", tag="xin")
            nc.sync.dma_start(h0[:], x_d[:, 0:512].bitcast(mmdt))

            # --- weights/bias: DMA once, one ACT-copy (f32r rounding +
            # makes matmul weight input ACT-produced). ---
            bias = wpool.tile([P, 3], f32)
            nc.sync.dma_start(bias[:], d_d[:])
            wraw = wpool.tile([P, 3 * P], f32, name="wraw", tag="wraw")
            for k in range(3):
                nc.sync.dma_start(wraw[:, k * P:(k + 1) * P], w_d[k])
            wall = wpool.tile([P, 3 * P], mmdt, name="wall", tag="wall")
            nc.scalar.activation(wall[:], wraw[:], Copy)
            wts = [wall[:, k * P:(k + 1) * P] for k in range(3)]

            # PE warmup burst: ~16 dense matmuls (~4us) to flip the HAM
            # clock gate to 2.4 GHz before the main chain; runs while the
            # input DMAs stream in.
            warm0 = wpool.tile([P, MM_F], f32, name="warm0", tag="warm0")
            nc.vector.memset(warm0[:], 0.0)
            warm_src = wpool.tile([P, MM_F], mmdt, name="warmsrc",
                                  tag="warmsrc")
            nc.scalar.activation(warm_src[:], warm0[:], Copy)
            warm_ps = psum.tile([P, MM_F], f32, name="warmps", tag="ps")
            for _ in range(16):
                nc.tensor.matmul(warm_ps[:], warm_src[:, 0:P], warm_src[:],
                                 start=True, stop=True)

            # Chunk-group software pipeline: within a group, consecutive
            # matmul groups come from rotating chunks, so each group's
            # dependency on the previous block's tanh has >=2 matmul
            # groups of slack and the PE streams. First chunk is small so
            # the chain starts as soon as its DMA lands.
            sizes = [512, 1536] + [CHUNK] * (NCHUNK - 1)
            offs = [sum(sizes[:i]) for i in range(len(sizes))]
            groups = [(0, 1, 2), (3, 4, 5)]
            hs = {}
            hs[0] = h0
            for ci in range(1, len(sizes)):
                h = xinpool.tile([P, sizes[ci]], mmdt, name=f"xin{ci}",
                                 tag=f"xin{min(ci, 2)}")
                nc.sync.dma_start(
                    h[:], x_d[:, offs[ci]:offs[ci] + sizes[ci]].bitcast(mmdt))
                hs[ci] = h
            for grp in groups:
                for kblk in range(6):
                    ks = WSETS[kblk]
                    last = kblk == 5
                    for ci in grp:
                        sz = sizes[ci]
                        ps = psum.tile([P, sz], f32,
                                       name=f"ps{ci}_{kblk}", tag="ps")
                        for j in range(0, sz, MM_F):
                            nc.tensor.matmul(
                                ps[:, j:j + MM_F],
                                wts[ks],
                                hs[ci][:, j:j + MM_F],
                                start=True, stop=True)
                        hn = (youtpool.tile([P, sz], f32,
                                            name=f"yo{ci}", tag=f"yo{ci}")
                              if last else
                              hpool.tile([P, sz], mmdt,
                                         name=f"h{ci}_{kblk}", tag="h"))
                        nc.scalar.activation(hn[:], ps[:], Tanh,
                                             bias=bias[:, ks:ks + 1],
                                             scale=1.0)
                        hs[ci] = hn
                        if last:
                            c0 = offs[ci]
                            nc.sync.dma_start(y_d[:, c0:c0 + sz], hn[:])
    nc.compile()
    return nc


def _film_params(c, Wk, bk, Wsk, bsk, Wbk, bbk):
    """A[b] = diag(scale[b]) @ Wk ; d[b] = scale[b]*bk + shift[b], float64."""
    c = c.astype(np.float64)
    scale = 1.0 / (1.0 + np.exp(-(c @ Wsk.astype(np.float64).T
                                  + bsk.astype(np.float64))))     # [B,3]
    shift = c @ Wbk.astype(np.float64).T + bbk.astype(np.float64)  # [B,3]
    A = scale[:, :, None] * Wk.astype(np.float64)[None]            # [B,3,3]
    d = scale * bk.astype(np.float64) + shift                      # [B,3]
    return A, d


def kernel(t, x, c,
           W0, b0, Ws0, bs0, Wb0, bb0,
           W1, b1, Ws1, bs1, Wb1, bb1,
           W2, b2, Ws2, bs2, Wb2, bb2):
    global LAST_EXEC_NS
    _install_profile_shim()
    from concourse.bass_utils import run_bass_kernel_spmd

    x = np.asarray(x)
    c = np.asarray(c)
    out_dtype = x.dtype

    if "nc" not in _CACHE:
        _CACHE["nc"] = _build_program()
    nc = _CACHE["nc"]

    # ---- host: FiLM affine params per (weight-set, batch), float64 ----
    sets = [
        _film_params(c, W0, b0, Ws0, bs0, Wb0, bb0),
        _film_params(c, W1, b1, Ws1, bs1, Wb1, bb1),
        _film_params(c, W2, b2, Ws2, bs2, Wb2, bb2),
    ]

    # ---- host: shard + relayout x ----
    # [B, N, 3] -> per core [128, L]: p = b*32 + comp*10 + g
    xp = np.zeros((B, NPAD, D), np.float32)
    xp[:, :N, :] = x
    # [B, 3, G, L]
    xt = np.ascontiguousarray(xp.transpose(0, 2, 1)).reshape(B, D, G, L)

    in_maps = []
    for cc in range(NCORES):
        bs = range(cc * BPC, (cc + 1) * BPC)
        X = np.zeros((BPC, 32, L), np.float32)
        for i, b in enumerate(bs):
            X[i, :30] = xt[b].reshape(30, L)
        W6 = np.zeros((3, P, P), np.float32)
        D128 = np.zeros((P, 3), np.float32)
        for k in range(3):
            A, dv = sets[k]
            for i, b in enumerate(bs):
                for ci_ in range(3):
                    for cj in range(3):
                        a = np.float32(A[b, ci_, cj])
                        for g in range(G):
                            W6[k, i * 32 + cj * G + g, i * 32 + ci_ * G + g] = a
                    D128[i * 32 + ci_ * G:i * 32 + ci_ * G + G, k] = \
                        np.float32(dv[b, ci_])
        in_maps.append({"x": X.reshape(P, L), "w": W6, "d": D128})

    res = run_bass_kernel_spmd(nc, in_maps, list(range(NCORES)),
                               trace=bool(PROFILE))
    if PROFILE:
        LAST_EXEC_NS = res.exec_time_ns

    # ---- host: gather + inverse layout ----
    out = np.empty((B, N, D), out_dtype)
    for cc in range(NCORES):
        Y = res.results[cc]["y"].reshape(BPC, 32, L)
        for i in range(BPC):
            b = cc * BPC + i
            # [30, L] -> [3, NPAD] -> [NPAD, 3] -> [:N]
            yb = Y[i, :30].reshape(D, NPAD)
            out[b] = yb.T[:N].astype(out_dtype, copy=False)
    return out


# revision 17
# speedup vs baseline: 1.0450x; 1.0450x over previous
"""Trainium2 Bass kernel for nn_FCond (FiLM-conditioned MLP chain).

Reference computation (B=32, N=100000, D=3, CDIM=128):
    h = x
    for kblk in [0, 1, 2, 2, 2, 2]:
        h = tanh((h @ Wk.T + bk) * sigmoid(c @ Wsk.T + bsk) + (c @ Wbk.T + bbk))

Since the FiLM conditioning depends only on (c, weights), each (batch,
block) reduces to an affine map  h' = tanh(A_kb @ h + d_kb)  with
A_kb [3,3], d_kb [3] precomputed on the host in float64.

Device strategy (pure data parallel over 8 cores, 4 batches/core):
  - Layout: partition p = b*32 + comp*10 + g  (4 batch-bands of 32
    partitions; 3 comps x 10 point-groups per band; rows 30,31 of each
    band are zero padding). Free dim = 10240 points per (b,comp,g)
    stream (N padded 100000 -> 102400).
  - Each block is ONE block-diagonal [128x128] matmul on TensorE
    (40 real points per column), PSUM accumulated, then ScalarE does
    tanh(psum + d) with a per-partition bias AP, evacuating PSUM->SBUF.
  - 5 chunks of 2048 columns stream through DMA-in -> 6 blocks -> DMA-out
    with double buffering.

MM_DTYPE: float32r (TF32-like PE mode, 1 cyc/col) vs float32 (exact,
4 cyc/col). Selected by MM_EXACT below.
"""
import sys
import types

import numpy as np

B, N, D, CDIM = 32, 100000, 3, 128
NCORES = 8
BPC = B // NCORES          # batches per core
G = 10                     # point-groups per (batch, comp)
L = 10240                  # points per partition stream (N padded / G)
NPAD = G * L               # padded N = 102400
P = 128                    # partitions
CHUNK = 2048               # free-dim chunk (4 PSUM banks fp32)
MM_F = 512                 # matmul free chunk (1 PSUM bank)
NCHUNK = L // CHUNK

MM_EXACT = False           # True -> float32 matmuls (exact, ~2.9x slower PE)

PROFILE = False            # set by test harness; collects HW exec time
LAST_EXEC_NS = None

_CACHE = {}


def _install_profile_shim():
    """Register the NTFF profile hook (missing antenv.axon_hooks in this
    container) so run_bass_kernel_spmd(trace=True) can report exec time."""
    if "antenv.axon_hooks" in sys.modules:
        return
    mod = types.ModuleType("antenv.axon_hooks")
    _state = {"hook": None}
    mod.set_axon_ntff_profile_hook = lambda h: _state.__setitem__("hook", h)
    mod.get_axon_ntff_profile_hook = lambda: _state["hook"]
    sys.modules["antenv.axon_hooks"] = mod
    try:
        from trn_agent_boot.trn_boot import _ntff_profile_via_ctypes
        mod.set_axon_ntff_profile_hook(
            _ntff_profile_via_ctypes("/opt/axon/libaxon_pjrt.so"))
    except Exception:
        pass
    import concourse.bass_utils as bu
    bu.upload_artifacts = lambda tmpdir: f"local:{tmpdir}"


def _build_program():
    import concourse.bacc as bacc
    import concourse.tile as tile
    from concourse import mybir

    f32 = mybir.dt.float32
    mmdt = f32 if MM_EXACT else mybir.dt.float32r
    Tanh = mybir.ActivationFunctionType.Tanh
    Copy = mybir.ActivationFunctionType.Copy
    WSETS = (0, 1, 2, 2, 2, 2)

    nc = bacc.Bacc("TRN2", target_bir_lowering=False, debug=False)
    x_d = nc.declare_dram_parameter("x", [P, L], f32, isOutput=False)
    w_d = nc.declare_dram_parameter("w", [3, P, P], f32, isOutput=False)
    d_d = nc.declare_dram_parameter("d", [P, 3], f32, isOutput=False)
    y_d = nc.declare_dram_parameter("y", [P, L], f32, isOutput=True)

    with tile.TileContext(nc) as tc:
        with (
            tc.tile_pool(name="wpool", bufs=1) as wpool,
            tc.tile_pool(name="xinpool", bufs=5) as xinpool,
            tc.tile_pool(name="youtpool", bufs=1) as youtpool,
            tc.tile_pool(name="hpool", bufs=4) as hpool,
            tc.tile_pool(name="psum", bufs=2, space="PSUM") as psum,
        ):
            # --- first compute chunk's DMA goes out before anything else
            # so the PE/ACT chain can start ASAP. ---
            h0 = xinpool.tile([P, CHUNK], mmdt, name="xin0", tag="xin")
            nc.sync.dma_start(h0[:], x_d[:, 0:CHUNK].bitcast(mmdt))

            # --- weights/bias: DMA once, one ACT-copy (f32r rounding +
            # makes matmul weight input ACT-produced). ---
            bias = wpool.tile([P, 3], f32)
            nc.sync.dma_start(bias[:], d_d[:])
            wraw = wpool.tile([P, 3 * P], f32, name="wraw", tag="wraw")
            for k in range(3):
                nc.sync.dma_start(wraw[:, k * P:(k + 1) * P], w_d[k])
            wall = wpool.tile([P, 3 * P], mmdt, name="wall", tag="wall")
            nc.scalar.activation(wall[:], wraw[:], Copy)
            wts = [wall[:, k * P:(k + 1) * P] for k in range(3)]

            # PE warmup burst: ~16 dense matmuls (~4us) to flip the HAM
            # clock gate to 2.4 GHz before the main chain; runs while the
            # input DMAs stream in.
            warm0 = wpool.tile([P, MM_F], f32, name="warm0", tag="warm0")
            nc.vector.memset(warm0[:], 0.0)
            warm_src = wpool.tile([P, MM_F], mmdt, name="warmsrc",
                                  tag="warmsrc")
            nc.scalar.activation(warm_src[:], warm0[:], Copy)
            warm_ps = psum.tile([P, MM_F], f32, name="warmps", tag="ps")
            for _ in range(16):
                nc.tensor.matmul(warm_ps[:], warm_src[:, 0:P], warm_src[:],
                                 start=True, stop=True)

            # Chunk-group software pipeline: within a group, consecutive
            # matmul groups come from rotating chunks, so each group's
            # dependency on the previous block's tanh has >=2 matmul
            # groups of slack and the PE streams. First chunk is small so
            # the chain starts as soon as its DMA lands.
            sizes = [CHUNK] * NCHUNK
            offs = [sum(sizes[:i]) for i in range(len(sizes))]
            groups = [(0, 1, 2), (3, 4)]
            hs = {}
            hs[0] = h0
            for ci in range(1, len(sizes)):
                h = xinpool.tile([P, sizes[ci]], mmdt, name=f"xin{ci}",
                                 tag="xin")
                nc.sync.dma_start(
                    h[:], x_d[:, offs[ci]:offs[ci] + sizes[ci]].bitcast(mmdt))
                hs[ci] = h
            for grp in groups:
                for kblk in range(6):
                    ks = WSETS[kblk]
                    last = kblk == 5
                    for ci in grp:
                        sz = sizes[ci]
                        ps = psum.tile([P, sz], f32,
                                       name=f"ps{ci}_{kblk}", tag="ps")
                        for j in range(0, sz, MM_F):
                            nc.tensor.matmul(
                                ps[:, j:j + MM_F],
                                wts[ks],
                                hs[ci][:, j:j + MM_F],
                                start=True, stop=True)
                        hn = (youtpool.tile([P, sz], f32,
                                            name=f"yo{ci}", tag=f"yo{ci}")
                              if last else
                              hpool.tile([P, sz], mmdt,
                                         name=f"h{ci}_{kblk}", tag="h"))
                        nc.scalar.activation(hn[:], ps[:], Tanh,
                                             bias=bias[:, ks:ks + 1],
                                             scale=1.0)
                        hs[ci] = hn
                        if last:
                            c0 = offs[ci]
                            nc.sync.dma_start(y_d[:, c0:c0 + sz], hn[:])
    nc.compile()
    return nc


def _film_params(c, Wk, bk, Wsk, bsk, Wbk, bbk):
    """A[b] = diag(scale[b]) @ Wk ; d[b] = scale[b]*bk + shift[b], float64."""
    c = c.astype(np.float64)
    scale = 1.0 / (1.0 + np.exp(-(c @ Wsk.astype(np.float64).T
                                  + bsk.astype(np.float64))))     # [B,3]
    shift = c @ Wbk.astype(np.float64).T + bbk.astype(np.float64)  # [B,3]
    A = scale[:, :, None] * Wk.astype(np.float64)[None]            # [B,3,3]
    d = scale * bk.astype(np.float64) + shift                      # [B,3]
    return A, d


def kernel(t, x, c,
           W0, b0, Ws0, bs0, Wb0, bb0,
           W1, b1, Ws1, bs1, Wb1, bb1,
           W2, b2, Ws2, bs2, Wb2, bb2):
    global LAST_EXEC_NS
    _install_profile_shim()
    from concourse.bass_utils import run_bass_kernel_spmd

    x = np.asarray(x)
    c = np.asarray(c)
    out_dtype = x.dtype

    if "nc" not in _CACHE:
        _CACHE["nc"] = _build_program()
    nc = _CACHE["nc"]

    # ---- host: FiLM affine params per (weight-set, batch), float64 ----
    sets = [
        _film_params(c, W0, b0, Ws0, bs0, Wb0, bb0),
        _film_params(c, W1, b1, Ws1, bs1, Wb1, bb1),
        _film_params(c, W2, b2, Ws2, bs2, Wb2, bb2),
    ]

    # ---- host: shard + relayout x ----
    # [B, N, 3] -> per core [128, L]: p = b*32 + comp*10 + g
    xp = np.zeros((B, NPAD, D), np.float32)
    xp[:, :N, :] = x
    # [B, 3, G, L]
    xt = np.ascontiguousarray(xp.transpose(0, 2, 1)).reshape(B, D, G, L)

    in_maps = []
    for cc in range(NCORES):
        bs = range(cc * BPC, (cc + 1) * BPC)
        X = np.zeros((BPC, 32, L), np.float32)
        for i, b in enumerate(bs):
            X[i, :30] = xt[b].reshape(30, L)
        W6 = np.zeros((3, P, P), np.float32)
        D128 = np.zeros((P, 3), np.float32)
        for k in range(3):
            A, dv = sets[k]
            for i, b in enumerate(bs):
                for ci_ in range(3):
                    for cj in range(3):
                        a = np.float32(A[b, ci_, cj])
                        for g in range(G):
                            W6[k, i * 32 + cj * G + g, i * 32 + ci_ * G + g] = a
                    D128[i * 32 + ci_ * G:i * 32 + ci_ * G + G, k] = \
                        np.float32(dv[b, ci_])
        in_maps.append({"x": X.reshape(P, L), "w": W6, "d": D128})

    res = run_bass_kernel_spmd(nc, in_maps, list(range(NCORES)),
                               trace=bool(PROFILE))
    if PROFILE:
        LAST_EXEC_NS = res.exec_time_ns

    # ---- host: gather + inverse layout ----
    out = np.empty((B, N, D), out_dtype)
    for cc in range(NCORES):
        Y = res.results[cc]["y"].reshape(BPC, 32, L)
        for i in range(BPC):
            b = cc * BPC + i
            # [30, L] -> [3, NPAD] -> [NPAD, 3] -> [:N]
            yb = Y[i, :30].reshape(D, NPAD)
            out[b] = yb.T[:N].astype(out_dtype, copy=False)
    return out


# revision 18
# speedup vs baseline: 1.0575x; 1.0119x over previous
"""Trainium2 Bass kernel for nn_FCond (FiLM-conditioned MLP chain).

Reference computation (B=32, N=100000, D=3, CDIM=128):
    h = x
    for kblk in [0, 1, 2, 2, 2, 2]:
        h = tanh((h @ Wk.T + bk) * sigmoid(c @ Wsk.T + bsk) + (c @ Wbk.T + bbk))

Since the FiLM conditioning depends only on (c, weights), each (batch,
block) reduces to an affine map  h' = tanh(A_kb @ h + d_kb)  with
A_kb [3,3], d_kb [3] precomputed on the host in float64.

Device strategy (pure data parallel over 8 cores, 4 batches/core):
  - Layout: partition p = b*32 + comp*10 + g  (4 batch-bands of 32
    partitions; 3 comps x 10 point-groups per band; rows 30,31 of each
    band are zero padding). Free dim = 10240 points per (b,comp,g)
    stream (N padded 100000 -> 102400).
  - Each block is ONE block-diagonal [128x128] matmul on TensorE
    (40 real points per column), PSUM accumulated, then ScalarE does
    tanh(psum + d) with a per-partition bias AP, evacuating PSUM->SBUF.
  - 5 chunks of 2048 columns stream through DMA-in -> 6 blocks -> DMA-out
    with double buffering.

MM_DTYPE: float32r (TF32-like PE mode, 1 cyc/col) vs float32 (exact,
4 cyc/col). Selected by MM_EXACT below.
"""
import sys
import types

import numpy as np

B, N, D, CDIM = 32, 100000, 3, 128
NCORES = 8
BPC = B // NCORES          # batches per core
G = 10                     # point-groups per (batch, comp)
L = 10240                  # points per partition stream (N padded / G)
NPAD = G * L               # padded N = 102400
P = 128                    # partitions
CHUNK = 2048               # free-dim chunk (4 PSUM banks fp32)
MM_F = 512                 # matmul free chunk (1 PSUM bank)
NCHUNK = L // CHUNK

MM_EXACT = False           # True -> float32 matmuls (exact, ~2.9x slower PE)

PROFILE = False            # set by test harness; collects HW exec time
LAST_EXEC_NS = None

_CACHE = {}


def _install_profile_shim():
    """Register the NTFF profile hook (missing antenv.axon_hooks in this
    container) so run_bass_kernel_spmd(trace=True) can report exec time."""
    if "antenv.axon_hooks" in sys.modules:
        return
    mod = types.ModuleType("antenv.axon_hooks")
    _state = {"hook": None}
    mod.set_axon_ntff_profile_hook = lambda h: _state.__setitem__("hook", h)
    mod.get_axon_ntff_profile_hook = lambda: _state["hook"]
    sys.modules["antenv.axon_hooks"] = mod
    try:
        from trn_agent_boot.trn_boot import _ntff_profile_via_ctypes
        mod.set_axon_ntff_profile_hook(
            _ntff_profile_via_ctypes("/opt/axon/libaxon_pjrt.so"))
    except Exception:
        pass
    import concourse.bass_utils as bu
    bu.upload_artifacts = lambda tmpdir: f"local:{tmpdir}"


def _build_program():
    import concourse.bacc as bacc
    import concourse.tile as tile
    from concourse import mybir

    f32 = mybir.dt.float32
    mmdt = f32 if MM_EXACT else mybir.dt.float32r
    Tanh = mybir.ActivationFunctionType.Tanh
    Copy = mybir.ActivationFunctionType.Copy
    WSETS = (0, 1, 2, 2, 2, 2)

    nc = bacc.Bacc("TRN2", target_bir_lowering=False, debug=False)
    x_d = nc.declare_dram_parameter("x", [P, L], f32, isOutput=False)
    w_d = nc.declare_dram_parameter("w", [3, P, P], f32, isOutput=False)
    d_d = nc.declare_dram_parameter("d", [P, 3], f32, isOutput=False)
    y_d = nc.declare_dram_parameter("y", [P, L], f32, isOutput=True)

    with tile.TileContext(nc) as tc:
        with (
            tc.tile_pool(name="wpool", bufs=1) as wpool,
            tc.tile_pool(name="xinpool", bufs=5) as xinpool,
            tc.tile_pool(name="youtpool", bufs=5) as youtpool,
            tc.tile_pool(name="hpool", bufs=6) as hpool,
            tc.tile_pool(name="psum", bufs=2, space="PSUM") as psum,
        ):
            # --- first compute chunk's DMA goes out before anything else
            # so the PE/ACT chain can start ASAP. ---
            h0 = xinpool.tile([P, CHUNK], mmdt, name="xin0", tag="xin")
            nc.sync.dma_start(h0[:], x_d[:, 0:CHUNK].bitcast(mmdt))

            # --- weights/bias: DMA once, one ACT-copy (f32r rounding +
            # makes matmul weight input ACT-produced). ---
            bias = wpool.tile([P, 3], f32)
            nc.sync.dma_start(bias[:], d_d[:])
            wraw = wpool.tile([P, 3 * P], f32, name="wraw", tag="wraw")
            for k in range(3):
                nc.sync.dma_start(wraw[:, k * P:(k + 1) * P], w_d[k])
            wall = wpool.tile([P, 3 * P], mmdt, name="wall", tag="wall")
            nc.scalar.activation(wall[:], wraw[:], Copy)
            wts = [wall[:, k * P:(k + 1) * P] for k in range(3)]

            # PE warmup burst: ~16 dense matmuls (~4us) to flip the HAM
            # clock gate to 2.4 GHz before the main chain; runs while the
            # input DMAs stream in.
            warm0 = wpool.tile([P, MM_F], f32, name="warm0", tag="warm0")
            nc.vector.memset(warm0[:], 0.0)
            warm_src = wpool.tile([P, MM_F], mmdt, name="warmsrc",
                                  tag="warmsrc")
            nc.scalar.activation(warm_src[:], warm0[:], Copy)
            warm_ps = psum.tile([P, MM_F], f32, name="warmps", tag="ps")
            for _ in range(16):
                nc.tensor.matmul(warm_ps[:], warm_src[:, 0:P], warm_src[:],
                                 start=True, stop=True)

            # Chunk-group software pipeline: within a group, consecutive
            # matmul groups come from rotating chunks, so each group's
            # dependency on the previous block's tanh has >=2 matmul
            # groups of slack and the PE streams. First chunk is small so
            # the chain starts as soon as its DMA lands.
            sizes = [CHUNK] * NCHUNK
            offs = [sum(sizes[:i]) for i in range(len(sizes))]
            groups = [(0, 1, 2), (3, 4)]
            hs = {}
            hs[0] = h0
            for grp in groups:
                for ci in grp:
                    if ci == 0:
                        continue
                    h = xinpool.tile([P, sizes[ci]], mmdt, name=f"xin{ci}",
                                     tag="xin")
                    nc.sync.dma_start(
                        h[:],
                        x_d[:, offs[ci]:offs[ci] + sizes[ci]].bitcast(mmdt))
                    hs[ci] = h
                for kblk in range(6):
                    ks = WSETS[kblk]
                    last = kblk == 5
                    for ci in grp:
                        sz = sizes[ci]
                        ps = psum.tile([P, sz], f32,
                                       name=f"ps{ci}_{kblk}", tag="ps")
                        for j in range(0, sz, MM_F):
                            nc.tensor.matmul(
                                ps[:, j:j + MM_F],
                                wts[ks],
                                hs[ci][:, j:j + MM_F],
                                start=True, stop=True)
                        hn = (youtpool.tile([P, sz], f32,
                                            name=f"yo{ci}", tag="yout")
                              if last else
                              hpool.tile([P, sz], mmdt,
                                         name=f"h{ci}_{kblk}", tag="h"))
                        nc.scalar.activation(hn[:], ps[:], Tanh,
                                             bias=bias[:, ks:ks + 1],
                                             scale=1.0)
                        hs[ci] = hn
                        if last:
                            c0 = offs[ci]
                            nc.sync.dma_start(y_d[:, c0:c0 + sz], hn[:])
    nc.compile()
    return nc


def _film_params(c, Wk, bk, Wsk, bsk, Wbk, bbk):
    """A[b] = diag(scale[b]) @ Wk ; d[b] = scale[b]*bk + shift[b], float64."""
    c = c.astype(np.float64)
    scale = 1.0 / (1.0 + np.exp(-(c @ Wsk.astype(np.float64).T
                                  + bsk.astype(np.float64))))     # [B,3]
    shift = c @ Wbk.astype(np.float64).T + bbk.astype(np.float64)  # [B,3]
    A = scale[:, :, None] * Wk.astype(np.float64)[None]            # [B,3,3]
    d = scale * bk.astype(np.float64) + shift                      # [B,3]
    return A, d


def kernel(t, x, c,
           W0, b0, Ws0, bs0, Wb0, bb0,
           W1, b1, Ws1, bs1, Wb1, bb1,
           W2, b2, Ws2, bs2, Wb2, bb2):
    global LAST_EXEC_NS
    _install_profile_shim()
    from concourse.bass_utils import run_bass_kernel_spmd

    x = np.asarray(x)
    c = np.asarray(c)
    out_dtype = x.dtype

    if "nc" not in _CACHE:
        _CACHE["nc"] = _build_program()
    nc = _CACHE["nc"]

    # ---- host: FiLM affine params per (weight-set, batch), float64 ----
    sets = [
        _film_params(c, W0, b0, Ws0, bs0, Wb0, bb0),
        _film_params(c, W1, b1, Ws1, bs1, Wb1, bb1),
        _film_params(c, W2, b2, Ws2, bs2, Wb2, bb2),
    ]

    # ---- host: shard + relayout x ----
    # [B, N, 3] -> per core [128, L]: p = b*32 + comp*10 + g
    xp = np.zeros((B, NPAD, D), np.float32)
    xp[:, :N, :] = x
    # [B, 3, G, L]
    xt = np.ascontiguousarray(xp.transpose(0, 2, 1)).reshape(B, D, G, L)

    in_maps = []
    for cc in range(NCORES):
        bs = range(cc * BPC, (cc + 1) * BPC)
        X = np.zeros((BPC, 32, L), np.float32)
        for i, b in enumerate(bs):
            X[i, :30] = xt[b].reshape(30, L)
        W6 = np.zeros((3, P, P), np.float32)
        D128 = np.zeros((P, 3), np.float32)
        for k in range(3):
            A, dv = sets[k]
            for i, b in enumerate(bs):
                for ci_ in range(3):
                    for cj in range(3):
                        a = np.float32(A[b, ci_, cj])
                        for g in range(G):
                            W6[k, i * 32 + cj * G + g, i * 32 + ci_ * G + g] = a
                    D128[i * 32 + ci_ * G:i * 32 + ci_ * G + G, k] = \
                        np.float32(dv[b, ci_])
        in_maps.append({"x": X.reshape(P, L), "w": W6, "d": D128})

    res = run_bass_kernel_spmd(nc, in_maps, list(range(NCORES)),
                               trace=bool(PROFILE))
    if PROFILE:
        LAST_EXEC_NS = res.exec_time_ns

    # ---- host: gather + inverse layout ----
    out = np.empty((B, N, D), out_dtype)
    for cc in range(NCORES):
        Y = res.results[cc]["y"].reshape(BPC, 32, L)
        for i in range(BPC):
            b = cc * BPC + i
            # [30, L] -> [3, NPAD] -> [NPAD, 3] -> [:N]
            yb = Y[i, :30].reshape(D, NPAD)
            out[b] = yb.T[:N].astype(out_dtype, copy=False)
    return out
